# revision 2
# baseline (speedup 1.0000x reference)
"""Trainium2 Bass kernel for nn_AttentionConvHead (windowed per-channel attention).

Math (per batch b, all channels d independent):
    Q = Wq @ q + bq ; K = Wk @ k + bk ; V = Wv @ v + bv        (1x1 convs)
    out[d,t,n] = sum_i softmax_i(Q[d,t,n] * Kpad[d,t+i,n]) * Vpad[d,t+i,n]
with K/V zero-padded by 3 on the time axis (pad contributes exp(0)=1 to the
softmax denominator and 0 to the numerator).

Distribution: pure data-parallel, one batch element per NeuronCore (B=8).

Per-core layout: partitions p = c + 64*g pack (channel, n-half); n (207,
padded to 208) splits into two groups of 104. Free dim is (t outer, n_local
inner) so a time shift is a contiguous free-dim offset of i*104.

Engine budget per core (the measured walls): DVE score-mul 3.5c/out +
value-mul 3.5c/out + recip/final ~2.1c/out ~= 131us; ACT exp 7el/out +
3el/out evictions ~= 124us; PE projections + 14 identity-matmul window
sums ~= 95us. v2 changes vs v1: uniform 1024-col chunks aligned to PSUM
banks (projection evictions consolidated 512->1024 cols, -12us ACT);
2-chunk pipeline skew between exp and value-mul so ACT latency never
stalls DVE; recip/final emitted ahead of the next chunk's den/num
matmuls so the PSUM WAR wait lands in PE slack.
"""

import numpy as np

B, C, T, N = 8, 64, 128, 207
D = 64
KS, PAD = 7, 3
NPAD, NG, P = 208, 104, 128
F = T * NG                 # 13312 free positions per partition
TP = T + 2 * PAD           # 134 padded time steps
FPAD = TP * NG             # 13936
MM = 512                   # psum bank = 512 fp32 matmul columns
CH = 1024                  # chunk size (free cols) for everything
NCH = F // CH              # 13 chunks
HEADP = PAD * NG           # 312 pad elements at each end of Kp/Vp

_CACHE = {}


def _build():
    import concourse.bacc as bacc
    import concourse.bass as bass
    import concourse.mybir as mybir
    from concourse.tile import TileContext

    f32 = mybir.dt.float32
    bf16 = mybir.dt.bfloat16
    AF = mybir.ActivationFunctionType

    nc = bacc.Bacc("TRN2", target_bir_lowering=False)

    xq = nc.declare_dram_parameter("xq", [P, F], bf16, isOutput=False)
    xk = nc.declare_dram_parameter("xk", [P, F], bf16, isOutput=False)
    xv = nc.declare_dram_parameter("xv", [P, F], bf16, isOutput=False)
    # wts: [wq | wk | wv | ident] as block-diag lhsT matrices, side by side
    wts = nc.declare_dram_parameter("wts", [P, 4 * P], bf16, isOutput=False)
    # bia: [bq | bk | bv] per-partition biases
    bia = nc.declare_dram_parameter("bia", [P, 3], f32, isOutput=False)
    out_d = nc.declare_dram_parameter("out", [P, F], bf16, isOutput=True)

    from contextlib import ExitStack

    with TileContext(nc) as tc, ExitStack() as ctx:
        consts = ctx.enter_context(tc.tile_pool(name="consts", bufs=1))
        xin = ctx.enter_context(tc.tile_pool(name="xin", bufs=4))
        big = ctx.enter_context(tc.tile_pool(name="big", bufs=1))
        qpool = ctx.enter_context(tc.tile_pool(name="qpool", bufs=3))
        spool = ctx.enter_context(tc.tile_pool(name="spool", bufs=3))
        wpool = ctx.enter_context(tc.tile_pool(name="wpool", bufs=2))
        npool = ctx.enter_context(tc.tile_pool(name="npool", bufs=2))
        outp = ctx.enter_context(tc.tile_pool(name="outp", bufs=2))
        # PSUM: projections rotate through 2x[128,1024] (4 banks);
        # den/num live in 1x[128,1024] each (4 banks). Total 8 banks.
        psA = ctx.enter_context(tc.tile_pool(name="psA", bufs=2, space="PSUM"))
        psD = ctx.enter_context(tc.tile_pool(name="psD", bufs=1, space="PSUM"))
        psN = ctx.enter_context(tc.tile_pool(name="psN", bufs=1, space="PSUM"))

        wts_s = consts.tile([P, 4 * P], bf16, tag="wts")
        bia_s = consts.tile([P, 3], f32, tag="bia")
        nc.sync.dma_start(out=wts_s, in_=wts.ap())
        nc.sync.dma_start(out=bia_s, in_=bia.ap())
        wq_s = wts_s[:, 0:P]
        wk_s = wts_s[:, P : 2 * P]
        wv_s = wts_s[:, 2 * P : 3 * P]
        id_s = wts_s[:, 3 * P : 4 * P]
        bq_s = bia_s[:, 0:1]
        bk_s = bia_s[:, 1:2]
        bv_s = bia_s[:, 2:3]

        Kp = big.tile([P, FPAD], bf16, tag="Kp")
        Vp = big.tile([P, FPAD], bf16, tag="Vp")

        nc.vector.memset(Kp[:, 0:HEADP], 0.0)
        nc.vector.memset(Kp[:, FPAD - HEADP : FPAD], 0.0)
        nc.vector.memset(Vp[:, 0:HEADP], 0.0)
        nc.vector.memset(Vp[:, FPAD - HEADP : FPAD], 0.0)

        def emit_A(k):
            """DMA + project + evict K,V chunk k ([k*CH, (k+1)*CH) interior)."""
            j0 = k * CH
            kt = xin.tile([P, CH], bf16, tag="xin")
            nc.sync.dma_start(out=kt, in_=xk.ap()[:, j0 : j0 + CH])
            vt = xin.tile([P, CH], bf16, tag="xin")
            nc.sync.dma_start(out=vt, in_=xv.ap()[:, j0 : j0 + CH])
            pk = psA.tile([P, CH], f32, tag="psA")
            for m0 in range(0, CH, MM):
                nc.tensor.matmul(
                    pk[:, m0 : m0 + MM], wk_s, kt[:, m0 : m0 + MM],
                    start=True, stop=True,
                )
            nc.scalar.activation(
                Kp[:, HEADP + j0 : HEADP + j0 + CH], pk, AF.Identity,
                bias=bk_s, scale=1.0,
            )
            pv = psA.tile([P, CH], f32, tag="psA")
            for m0 in range(0, CH, MM):
                nc.tensor.matmul(
                    pv[:, m0 : m0 + MM], wv_s, vt[:, m0 : m0 + MM],
                    start=True, stop=True,
                )
            nc.scalar.activation(
                Vp[:, HEADP + j0 : HEADP + j0 + CH], pv, AF.Identity,
                bias=bv_s, scale=1.0,
            )

        def emit_Q(c):
            """Stream xq chunk c in, project, evict (+bias) to bf16 qb."""
            b = c * CH
            qx = qpool.tile([P, CH], bf16, tag="qx")
            nc.sync.dma_start(out=qx, in_=xq.ap()[:, b : b + CH])
            pq = psA.tile([P, CH], f32, tag="psA")
            for m0 in range(0, CH, MM):
                nc.tensor.matmul(
                    pq[:, m0 : m0 + MM], wq_s, qx[:, m0 : m0 + MM],
                    start=True, stop=True,
                )
            qb = qpool.tile([P, CH], bf16, tag="qb")
            nc.scalar.activation(qb, pq, AF.Identity, bias=bq_s, scale=1.0)
            return qb

        def emit_score(c, qb):
            """Batched score mul (all 7 windows, one op) + 2-op exp."""
            b = c * CH
            seb = spool.tile([P, KS, CH], bf16, tag="seb")
            qb_b = bass.AP(
                tensor=qb.tensor, offset=qb.offset, ap=[qb.ap[0], [0, KS], [1, CH]]
            )
            kp_v = bass.AP(
                tensor=Kp.tensor, offset=Kp.offset + b, ap=[Kp.ap[0], [NG, KS], [1, CH]]
            )
            nc.vector.tensor_mul(seb, qb_b, kp_v)
            nc.scalar.activation(seb[:, 0:4, :], seb[:, 0:4, :], AF.Exp)
            nc.scalar.activation(seb[:, 4:KS, :], seb[:, 4:KS, :], AF.Exp)
            return seb

        def emit_value(c, seb):
            """Value products for chunk c (exp must be 2 chunks back)."""
            b = c * CH
            wb7 = wpool.tile([P, KS, CH], bf16, tag="wb7")
            vp_a = bass.AP(
                tensor=Vp.tensor, offset=Vp.offset + b, ap=[Vp.ap[0], [NG, 4], [1, CH]]
            )
            vp_b = bass.AP(
                tensor=Vp.tensor,
                offset=Vp.offset + b + 4 * NG,
                ap=[Vp.ap[0], [NG, KS - 4], [1, CH]],
            )
            nc.vector.tensor_mul(wb7[:, 0:4, :], seb[:, 0:4, :], vp_a)
            nc.vector.tensor_mul(wb7[:, 4:KS, :], seb[:, 4:KS, :], vp_b)
            return wb7

        def emit_sums(c, seb, wb7):
            """PSUM window accumulation for chunk c -> (den, num) tiles."""
            den = psD.tile([P, CH], f32, tag="den")
            num = psN.tile([P, CH], f32, tag="num")
            for i in range(KS):
                first, last = i == 0, i == KS - 1
                for m0 in range(0, CH, MM):
                    nc.tensor.matmul(
                        den[:, m0 : m0 + MM],
                        id_s,
                        seb[:, i, m0 : m0 + MM],
                        start=first, stop=last, skip_group_check=True,
                    )
                    nc.tensor.matmul(
                        num[:, m0 : m0 + MM],
                        id_s,
                        wb7[:, i, m0 : m0 + MM],
                        start=first, stop=last, skip_group_check=True,
                    )
            return den, num

        def emit_norm(c, den, num):
            """Normalize chunk c and DMA out."""
            b = c * CH
            r = outp.tile([P, CH], f32, tag="r")
            nc.vector.reciprocal_approx_fast(out=r, in_=den)
            ot = outp.tile([P, CH], bf16, tag="ot")
            nc.vector.tensor_mul(ot, num, r)
            nc.sync.dma_start(out=out_d.ap()[:, b : b + CH], in_=ot)

        # Software pipeline, skew 2 between exp and value, 3 to normalize:
        #   iter c emits: A(c+1), Q(c), score+exp(c), value(c-2),
        #                 norm(c-3), sums(c-2)
        # norm(c-3) is emitted before sums(c-2) so den/num's WAR wait on the
        # previous recip/final lands inside PE slack, not on the DVE path.
        qbs, sebs, wbs, dn = {}, {}, {}, {}
        ai = 0
        for c in range(NCH + 3):
            if c < NCH:
                need = c + 2 if c + 1 < NCH else NCH  # A-steps needed
                while ai < min(need, NCH):
                    emit_A(ai)
                    ai += 1
                qbs[c] = emit_Q(c)
                sebs[c] = emit_score(c, qbs.pop(c))
            if 0 <= c - 2 < NCH:
                wbs[c - 2] = emit_value(c - 2, sebs[c - 2])
            if 0 <= c - 3 < NCH:
                emit_norm(c - 3, *dn.pop(c - 3))
            if 0 <= c - 2 < NCH:
                dn[c - 2] = emit_sums(c - 2, sebs.pop(c - 2), wbs.pop(c - 2))

    nc.compile()
    return nc


def _pack(x, bf):
    """[C, T, N] f32 -> [128, F] bf16: partition p = c + 64g, free = t*104 + n_loc."""
    xp = np.zeros((C, T, NPAD), np.float32)
    xp[:, :, :N] = x
    return np.ascontiguousarray(
        xp.reshape(C, T, 2, NG).transpose(2, 0, 1, 3).reshape(P, F)
    ).astype(bf)


def _unpack(o):
    """[128, F] -> [D, T, N]."""
    o = np.asarray(o, np.float32)
    return np.ascontiguousarray(
        o.reshape(2, D, T, NG).transpose(1, 2, 0, 3).reshape(D, T, NPAD)[:, :, :N]
    )


def _lhsT_blockdiag(W):
    Z = np.zeros((P, P), np.float32)
    Z[:C, :D] = W.T
    Z[C:, D:] = W.T
    return Z


def _prep_in_maps(q, k, v, Wq, bq, Wk, bk, Wv, bv):
    import ml_dtypes

    bf = ml_dtypes.bfloat16
    wts = np.concatenate(
        [
            _lhsT_blockdiag(np.asarray(Wq, np.float32)),
            _lhsT_blockdiag(np.asarray(Wk, np.float32)),
            _lhsT_blockdiag(np.asarray(Wv, np.float32)),
            np.eye(P, dtype=np.float32),
        ],
        axis=1,
    ).astype(bf)
    bia = np.stack(
        [np.concatenate([b, b]) for b in (bq, bk, bv)], axis=1
    ).astype(np.float32)
    in_maps = []
    for b in range(B):
        in_maps.append(
            {
                "xq": _pack(np.asarray(q[b], np.float32), bf),
                "xk": _pack(np.asarray(k[b], np.float32), bf),
                "xv": _pack(np.asarray(v[b], np.float32), bf),
                "wts": wts,
                "bia": bia,
            }
        )
    return in_maps


def run(inputs, trace=False):
    """Build (cached), run on 8 cores, return (output, BassKernelResults)."""
    from concourse.bass_utils import run_bass_kernel_spmd

    if "nc" not in _CACHE:
        _CACHE["nc"] = _build()
    nc = _CACHE["nc"]
    in_maps = _prep_in_maps(**inputs)
    res = run_bass_kernel_spmd(nc, in_maps, core_ids=list(range(B)), trace=trace)
    out = np.stack([_unpack(np.asarray(res.results[b]["out"])) for b in range(B)])
    return out, res


def kernel(q, k, v, Wq, bq, Wk, bk, Wv, bv):
    out, _ = run(dict(q=q, k=k, v=v, Wq=Wq, bq=bq, Wk=Wk, bk=bk, Wv=Wv, bv=bv))
    return out
